# revision 1
# baseline (speedup 1.0000x reference)
"""GCNEncoder Trainium2 kernel (8 NeuronCores, SPMD).

Strategy (graph/data parallel, per sharding hint):
  - Nodes are dealt round-robin-by-degree across 8 cores (2500 each); the
    [H,H] weights are replicated.
  - Per GCN layer: each core scales its node rows by dinv=1/sqrt(deg), casts
    to bf16 and AllGathers the full 20000x256 "table" into every core's HBM.
  - Message aggregation = segment-sum over in-edges:  per 128-destination
    group, a transposed dma_gather pulls the source rows (feature-major:
    [128h, 2, 128*K]) and a strided DVE reduce sums each destination's K
    slots (padding slots point at an all-zero table row).
  - The GCNConv reorder agg(x) @ W == agg(x @ W) lets one aggregation per
    layer feed the [HxH] matmul afterwards; out2/out3 share the layer-3
    aggregation.  norm = dinv[row]*dinv[col] factorizes into the table
    pre-scale and a per-destination post-scale fused into the PSUM->SBUF
    activation (bias is added via a K=1 rank-1 matmul of sqrt(deg) x b).

Self-contained: hardcodes the problem shapes; only needs numpy + concourse.
"""

import numpy as np

# -------------------- problem constants --------------------
N_NODES = 20000
N_EDGES = 320000
H = 256
C = 8  # cores

_KERNEL_CACHE = {}
LAST_RESULTS = None  # BassKernelResults of the most recent run (for profiling)


# -------------------- host-side graph prep --------------------
def _prep_graph(edge_index, n_nodes, n_cores):
    """Partition nodes, build per-core padded gather-slot index arrays.

    Returns dict with permutation, per-core degree arrays, gather indices.
    """
    P = n_nodes // n_cores  # nodes per core
    row = edge_index[0].astype(np.int64)
    col = edge_index[1].astype(np.int64)
    loop = np.arange(n_nodes, dtype=np.int64)
    row_f = np.concatenate([row, loop])
    col_f = np.concatenate([col, loop])
    deg = np.bincount(col_f, minlength=n_nodes).astype(np.int64)  # >= 1

    # deal nodes round-robin by ascending degree -> every core gets an
    # almost identical degree profile, sorted ascending within the core.
    order = np.argsort(deg, kind="stable")
    pos = np.empty(n_nodes, dtype=np.int64)
    pos[order] = np.arange(n_nodes)
    new_id = (pos % n_cores) * P + pos // n_cores  # old -> new
    orig_of_new = np.empty(n_nodes, dtype=np.int64)
    orig_of_new[new_id] = np.arange(n_nodes)

    src_new = new_id[row_f]
    dst_new = new_id[col_f]

    PT = ((P + 127) // 128) * 128  # padded dest count per core
    NG = PT // 128  # 128-dest groups
    PR = P + 16  # table rows contributed per rank (16 zero pad rows)
    ZROW = P  # rank0's first pad row: an all-zero table row

    deg_new = deg[orig_of_new]  # per new id
    # per-core local degree arrays, padded with 1.0
    deg_loc = np.ones((n_cores, PT), dtype=np.float32)
    for c in range(n_cores):
        deg_loc[c, :P] = deg_new[c * P : (c + 1) * P]

    # max (over cores) degree within a local-dest range, %4-rounded
    # (clean 2x-mode DVE pair-adds need K%4: even halves at 4B alignment)
    def range_K(lo, hi):
        m = 0
        for c in range(n_cores):
            a, b = c * P + lo, c * P + min(hi, P)
            if a < b:
                m = max(m, int(deg_new[a:b].max()))
        return max(((m + 3) // 4) * 4, 4)

    # gather chunks: whole-dest sub-ranges of each 128-dest group, <= MAXI
    # indices per dma_gather (descriptor-ring limit at 48KB DMA scratch);
    # each chunk gets its own K to minimise zero-slot padding
    MAXI = 2944
    Kg = []
    chunks = []  # (dest_off_in_core, dc, K, idx_off)
    ioff = 0
    for g in range(NG):
        gK = range_K(g * 128, (g + 1) * 128)
        Kg.append(gK)
        dc = 128
        while dc * gK > MAXI:
            dc //= 2
        assert dc >= 32
        for d0 in range(0, 128, dc):
            cK = range_K(g * 128 + d0, g * 128 + d0 + dc)
            assert (dc * cK) % 128 == 0
            chunks.append((g * 128 + d0, dc, cK, ioff))
            ioff += dc * cK
    TOT = int(ioff)  # slots per core (same for all cores)

    # per-dest slot base/K for filling
    dest_base = np.zeros(PT, dtype=np.int64)
    dest_K = np.ones(PT, dtype=np.int64)
    for doff, dc, cK, io in chunks:
        d = np.arange(dc)
        dest_base[doff : doff + dc] = io + d * cK
        dest_K[doff : doff + dc] = cK

    # slot array [cores, TOT] filled with ZROW, then scatter edge sources.
    # table row of new node id n = (n // P) * PR + (n % P)
    src_trow = (src_new // P) * PR + (src_new % P)
    slots = np.full((n_cores, TOT), ZROW, dtype=np.int64)
    e_core = dst_new // P
    e_dloc = dst_new % P
    sort_k = np.argsort(e_core * n_nodes + e_dloc, kind="stable")
    sc, sd, ss = e_core[sort_k], e_dloc[sort_k], src_trow[sort_k]
    # rank within each (core,dest) run
    key = sc * n_nodes + sd
    first = np.r_[True, key[1:] != key[:-1]]
    run_start = np.maximum.accumulate(np.where(first, np.arange(key.size), 0))
    rank = np.arange(key.size) - run_start
    flat = dest_base[sd] + rank
    slots[sc, flat] = ss

    # wrap to [128, TOT//16] int16: element (p, s) = slots[s*16 + p%16]
    assert TOT % 16 == 0
    wrapped = np.empty((n_cores, 128, TOT // 16), dtype=np.int16)
    for c in range(n_cores):
        w16 = slots[c].reshape(TOT // 16, 16).T.astype(np.int16)  # [16, TOT/16]
        wrapped[c] = np.tile(w16, (8, 1))

    return dict(
        P=P, PT=PT, NG=NG, TOT=TOT, ZROW=ZROW, PR=PR,
        Kg=[int(k) for k in Kg], offs=[0],
        chunks=chunks,
        new_id=new_id, orig_of_new=orig_of_new,
        deg_loc=deg_loc, gidx=wrapped,
    )


# -------------------- bass kernel builder --------------------
def _build_bass(n_nodes, n_cores, h, P, PT, NG, TOT, Kg, offs, PR, chunks,
                repeat=1, collective=True):
    import concourse.bass as bass
    import concourse.bacc as bacc
    import concourse.mybir as mybir
    import concourse.tile as tile
    from concourse import library_config

    dt = mybir.dt
    f32, bf16, i16 = dt.float32, dt.bfloat16, dt.int16
    AF = mybir.ActivationFunctionType
    NT = PT // 128  # node tiles per core
    NTAB = n_cores * PR  # table rows (rank r at [r*PR, r*PR+P); pads zero)
    KC = h // 128  # contraction chunks (2)

    nc = bacc.Bacc(dynamic_dma_scratch_size=49152)
    x_in = nc.declare_dram_parameter("x_shard", [P, h], f32, isOutput=False)
    deg_in = nc.declare_dram_parameter("deg_loc", [PT], f32, isOutput=False)
    idx_in = nc.declare_dram_parameter("gidx", [128, TOT // 16], i16, isOutput=False)
    W_in = [nc.declare_dram_parameter(nm, [h, h], f32, isOutput=False)
            for nm in ("W1", "W1_1", "W2", "W3")]
    b_in = [nc.declare_dram_parameter(nm, [h], f32, isOutput=False)
            for nm in ("b1", "b1_1", "b2", "b3")]
    out2_ext = nc.declare_dram_parameter("out2", [P, h], f32, isOutput=True)
    out3_ext = nc.declare_dram_parameter("out3", [P, h], f32, isOutput=True)

    with tile.TileContext(nc) as tc:
        with (
            tc.tile_pool(name="dram", bufs=1, space="DRAM") as dpool,
            tc.tile_pool(name="const", bufs=1) as cpool,
            tc.tile_pool(name="gather", bufs=4) as gpool,
            tc.tile_pool(name="rbuf", bufs=6) as rpool,
            tc.tile_pool(name="work", bufs=4) as wpool,
            tc.tile_pool(name="psum", bufs=8, space="PSUM") as ppool,
        ):
            # ---- internal DRAM ---- (per-repeat for benchmark variants:
            # Tile requires a single writer for Shared DRAM)
            ag_in_r = [
                [dpool.tile([PR, h], bf16, name=f"agin{L}_{r}") for L in range(3)]
                for r in range(repeat)
            ]
            if collective:
                tables_r = [
                    [dpool.tile([NTAB, h], bf16, addr_space="Shared",
                                name=f"table{L}_{r}") for L in range(3)]
                    for r in range(repeat)
                ]
            else:  # timing-study variant: tables fed as plain inputs, no AG
                tin = [
                    nc.declare_dram_parameter(f"tbl{L}", [NTAB, h], bf16,
                                              isOutput=False)
                    for L in range(3)
                ]
                tables_r = [tin for _ in range(repeat)]

            # ---- constants ----
            w_sb = []
            for i in range(4):
                wt = cpool.tile([128, KC, h], f32, name=f"w{i}")
                nc.sync.dma_start(wt[:], W_in[i].rearrange("(c p) j -> p c j", p=128))
                wb = cpool.tile([128, KC, h], bf16, name=f"wb{i}")
                nc.vector.tensor_copy(wb[:], wt[:])
                w_sb.append(wb)
            b_sb = []
            for i in range(4):
                bt = cpool.tile([1, h], f32, name=f"bv{i}")
                nc.sync.dma_start(bt[:], b_in[i][None, :])
                b_sb.append(bt)

            deg_row = cpool.tile([1, PT], f32, name="deg_row")
            nc.sync.dma_start(deg_row[:], deg_in[None, :])
            sqd_row = cpool.tile([1, PT], f32, name="sqd_row")
            nc.scalar.sqrt(sqd_row[:], deg_row[:])

            deg_nm = cpool.tile([128, NT], f32, name="deg_nm")
            nc.sync.dma_start(deg_nm[:], deg_in.rearrange("(t p) -> p t", p=128))
            sq_nm = cpool.tile([128, NT], f32, name="sq_nm")
            nc.scalar.sqrt(sq_nm[:], deg_nm[:])
            dinv_nm = cpool.tile([128, NT], f32, name="dinv_nm")
            nc.vector.reciprocal(dinv_nm[:], sq_nm[:])
            dinv2_nm = cpool.tile([128, NT], f32, name="dinv2_nm")
            nc.vector.tensor_mul(dinv2_nm[:], dinv_nm[:], dinv_nm[:])

            gidx = cpool.tile([128, TOT // 16], i16, name="gidx_sb")
            nc.sync.dma_start(gidx[:], idx_in[:])

            rg = [list(range(n_cores))]
            zpad = cpool.tile([PR - P, h], bf16, name="zpad")
            nc.vector.memset(zpad[:], 0.0)

            # chunks grouped by 128-dest tile
            by_group = [[] for _ in range(NG)]
            for ch in chunks:
                by_group[ch[0] // 128].append(ch)

            def mm_into(ps, Rb, t, wi, start=True):
                for c in range(KC):
                    nc.tensor.matmul(
                        ps[:],
                        lhsT=Rb[:, c, :],
                        rhs=w_sb[wi][:, c, :],
                        start=(start and c == 0),
                        stop=False,
                    )
                nc.tensor.matmul(
                    ps[:],
                    lhsT=sqd_row[0:1, t * 128 : (t + 1) * 128],
                    rhs=b_sb[wi][:],
                    start=False,
                    stop=True,
                )

            def process_layer(rep, L):
                """AllGather table L, then per 128-dest group: gather in-edge
                rows, tree-reduce on DVE, matmul + fused epilogue, emit either
                the next layer's AG input (L<2) or the two output heads."""
                ag_in = ag_in_r[rep]
                if collective:
                    nc.gpsimd.collective_compute(
                        "AllGather",
                        mybir.AluOpType.bypass,
                        replica_groups=rg,
                        ins=[ag_in[L].opt()],
                        outs=[tables_r[rep][L].opt()],
                    )
                # biggest groups first: the layer tail (which gates the next
                # AllGather) then drains through the cheapest chunks
                for g in sorted(range(NG), key=lambda gg: -Kg[gg]):
                    Rg = rpool.tile([128, KC, 128], f32, tag="Rg",
                                    name=f"Rg{rep}_{L}_{g}")
                    for ci, (doff, dc, K, ioff) in enumerate(by_group[g]):
                        n_idx = dc * K
                        gt = gpool.tile([128, KC, n_idx], bf16, tag="gt",
                                        name=f"gt{rep}_{L}_{g}_{ci}")
                        nc.gpsimd.dma_gather(
                            gt[:],
                            tables_r[rep][L][:, :],
                            gidx[:, ioff // 16 : (ioff + n_idx) // 16],
                            n_idx,
                            n_idx,
                            h,
                            transpose=True,
                            single_packet=(n_idx <= 896),
                        )
                        # in-place pair-add halving while 2x-mode legal
                        # (runs even + 4B-aligned bases requires K' % 4 == 0)
                        cK = K
                        g4 = gt.rearrange("p c (d k) -> p c d k", k=K)
                        while cK % 4 == 0 and cK > 2:
                            nh = cK // 2
                            nc.vector.tensor_add(
                                g4[:, :, :, 0:nh],
                                g4[:, :, :, 0:nh],
                                g4[:, :, :, nh:cK],
                            )
                            cK = nh
                        nc.vector.tensor_reduce(
                            Rg[:, :, doff % 128 : doff % 128 + dc],
                            g4[:, :, :, 0:cK],
                            axis=mybir.AxisListType.X,
                            op=mybir.AluOpType.add,
                        )
                    Rb = rpool.tile([128, KC, 128], bf16, tag="Rbg",
                                    name=f"Rb{rep}_{L}_{g}")
                    nc.scalar.copy(Rb[:], Rg[:])  # ACT: f32 -> bf16 for the PE
                    rows = min(128, P - g * 128)
                    if L < 2:
                        ps = ppool.tile([128, h], f32, tag="ps",
                                        name=f"ps{rep}_{L}_{g}")
                        mm_into(ps, Rb, g, L)
                        # T = dinv*relu(dinv*(RW) + b) = relu(dinv^2*psum)
                        tt = wpool.tile([128, h], bf16, tag="tt",
                                        name=f"ttl{rep}_{L}_{g}")
                        nc.scalar.activation(
                            tt[:], ps[:], AF.Relu, scale=dinv2_nm[:, g : g + 1]
                        )
                        nc.sync.dma_start(
                            ag_in[L + 1][g * 128 : g * 128 + rows, :], tt[:rows, :]
                        )
                    else:
                        ps2 = ppool.tile([128, h], f32, tag="ps",
                                         name=f"ps2_{rep}_{g}")
                        mm_into(ps2, Rb, g, 2)
                        ps3 = ppool.tile([128, h], f32, tag="ps",
                                         name=f"ps3_{rep}_{g}")
                        mm_into(ps3, Rb, g, 3)
                        o2 = wpool.tile([128, h], f32, tag="hsb",
                                        name=f"o2_{rep}_{g}")
                        nc.scalar.activation(
                            o2[:], ps2[:], AF.Copy, scale=dinv_nm[:, g : g + 1]
                        )
                        nc.sync.dma_start(
                            out2_ext[g * 128 : g * 128 + rows, :], o2[:rows, :]
                        )
                        o3 = wpool.tile([128, h], f32, tag="hsb",
                                        name=f"o3_{rep}_{g}")
                        nc.scalar.activation(
                            o3[:], ps3[:], AF.Copy, scale=dinv_nm[:, g : g + 1]
                        )
                        nc.sync.dma_start(
                            out3_ext[g * 128 : g * 128 + rows, :], o3[:rows, :]
                        )

            for rep in range(repeat):
                ag_in = ag_in_r[rep]
                for L in range(3):
                    nc.sync.dma_start(ag_in[L][P:PR, :], zpad[:])

                # ---- prologue: T1 = bf16(dinv * x) on ACT ----
                for t in range(NT):
                    rows = min(128, P - t * 128)
                    xt = wpool.tile([128, h], f32, tag="xt", name=f"xt{rep}_{t}")
                    nc.sync.dma_start(xt[:rows, :], x_in[t * 128 : t * 128 + rows, :])
                    tt = wpool.tile([128, h], bf16, tag="tt", name=f"tt{rep}_{t}")
                    nc.scalar.activation(
                        tt[:rows, :], xt[:rows, :], AF.Copy,
                        scale=dinv_nm[:rows, t : t + 1],
                    )
                    nc.sync.dma_start(
                        ag_in[0][t * 128 : t * 128 + rows, :], tt[:rows, :]
                    )

                for L in range(3):
                    process_layer(rep, L)

    nc.compile()
    return nc


# -------------------- public entry --------------------
def kernel(x, edge_index, W1, b1, W1_1, b1_1, W2, b2, W3, b3):
    from concourse.bass_utils import run_bass_kernel_spmd

    x = np.asarray(x, dtype=np.float32)
    edge_index = np.asarray(edge_index, dtype=np.int32)
    n_nodes, h = x.shape
    meta = _prep_graph(edge_index, n_nodes, C)
    P, PT, NG, TOT = meta["P"], meta["PT"], meta["NG"], meta["TOT"]

    key = (n_nodes, h, tuple(meta["Kg"]))
    if key not in _KERNEL_CACHE:
        _KERNEL_CACHE[key] = _build_bass(
            n_nodes, C, h, P, PT, NG, TOT, meta["Kg"], meta["offs"], meta["PR"],
            meta["chunks"],
        )
    nc = _KERNEL_CACHE[key]

    oon = meta["orig_of_new"]
    Ws = {"W1": W1, "W1_1": W1_1, "W2": W2, "W3": W3}
    bs = {"b1": b1, "b1_1": b1_1, "b2": b2, "b3": b3}
    in_maps = []
    for c in range(C):
        m = {
            "x_shard": np.ascontiguousarray(
                x[oon[c * P : (c + 1) * P]], dtype=np.float32
            ),
            "deg_loc": meta["deg_loc"][c],
            "gidx": np.ascontiguousarray(meta["gidx"][c]),
        }
        for k, v in Ws.items():
            m[k] = np.ascontiguousarray(v, dtype=np.float32)
        for k, v in bs.items():
            m[k] = np.ascontiguousarray(v, dtype=np.float32)
        in_maps.append(m)

    global LAST_RESULTS
    LAST_RESULTS = run_bass_kernel_spmd(nc, in_maps, core_ids=list(range(C)))
    res = LAST_RESULTS.results

    out2_new = np.concatenate([res[c]["out2"] for c in range(C)], axis=0)
    out3_new = np.concatenate([res[c]["out3"] for c in range(C)], axis=0)
    new_id = meta["new_id"]
    return out2_new[new_id].astype(np.float32), out3_new[new_id].astype(np.float32)



# revision 5
# speedup vs baseline: 1.2075x; 1.2075x over previous
"""GCNEncoder Trainium2 kernel (8 NeuronCores, SPMD).

Strategy (graph/data parallel, per sharding hint):
  - Nodes sorted by in-degree and chopped into 128-node blocks; blocks are
    dealt to the 8 cores so every core sees the same block-K profile (one
    compiled kernel serves all cores; only index/feature data differs).
  - Layer-0 "table" (dinv-scaled bf16 x, padded, replicated) is a host input;
    layers 1/2 tables are produced on device and AllGathered (2 collectives).
  - Per GCN layer: per 128-destination block, a transposed dma_gather pulls
    the in-edge source rows k-major ([128 feat, 2, K, 128 dest]); an in-place
    DVE pair-add tree reduces over K, ending in a bf16 slab that is used
    directly as matmul lhsT.  Self-loop terms never enter the gather: the
    core's own (feature-major) table tile is DVE-added into the slab; that
    tile is a host input for layer 0 and produced by PE transposes of the
    epilogue tiles for layers 1/2.
  - The GCNConv reorder agg(x) @ W == agg(x @ W) lets one aggregation per
    layer feed the [HxH] matmul; out2/out3 share the layer-3 aggregation.
    norm = dinv[row]*dinv[col] factorizes into the table pre-scale and a
    per-destination post-scale fused into the PSUM->SBUF activation (bias is
    added via a K=1 rank-1 matmul of sqrt(deg) x b).  Outputs leave the
    device in bf16 and are cast to f32 on the host.

Self-contained: hardcodes the problem shapes; only needs numpy + concourse.
"""

import numpy as np
import ml_dtypes

# -------------------- problem constants --------------------
N_NODES = 20000
N_EDGES = 320000
H = 256
C = 8  # cores

_KERNEL_CACHE = {}
LAST_RESULTS = None  # BassKernelResults of the most recent run (for profiling)

BF16 = ml_dtypes.bfloat16


# -------------------- host-side graph prep --------------------
def _prep_graph(edge_index, n_nodes, n_cores):
    """Partition nodes into degree-sorted 128-blocks dealt across cores;
    build per-core k-major gather-slot arrays with identical shapes.
    """
    row = edge_index[0].astype(np.int64)
    col = edge_index[1].astype(np.int64)
    deg_in = np.bincount(col, minlength=n_nodes).astype(np.int64)
    deg_full = deg_in + 1  # self loop
    dinv = 1.0 / np.sqrt(deg_full.astype(np.float64))

    NBS = 20  # block slots per core
    NB = n_cores * NBS  # 160 block slots (157 real blocks + 3 empty)
    PT = NBS * 128  # 2560 local dest slots per core
    PR = PT + 16  # table rows per rank (16 zero pad rows)
    ZROW = PT  # rank0's first pad row: all-zero table row
    NTAB = n_cores * PR

    order = np.argsort(deg_in, kind="stable")  # ascending degree
    # block b (sorted ascending) holds nodes order[b*128:(b+1)*128].
    # Deal: sort blocks descending, slot s of every core gets one of blocks
    # [8s, 8s+8); slot-K := max K in the window so all cores share shapes.
    nreal = (n_nodes + 127) // 128  # 157
    blocks = list(range(nreal - 1, -1, -1)) + [-1] * (NB - nreal)  # K desc
    slot_blocks = [blocks[s * n_cores:(s + 1) * n_cores] for s in range(NBS)]

    def range_K(s, d0, dc):
        """Max in-degree over lanes [d0, d0+dc) across the slot's blocks."""
        m = 0
        for b in slot_blocks[s]:
            if b < 0:
                continue
            lo = b * 128 + d0
            hi = min(lo + dc, n_nodes)
            if lo < hi:
                m = max(m, int(deg_in[order[lo:hi]].max()))
        return max(m, 1)

    slot_K = [range_K(s, 0, 128) for s in range(NBS)]

    # node -> (core, local): block slot_blocks[s][c] -> core c, slot s
    core_of = np.empty(n_nodes, dtype=np.int64)
    loc_of = np.empty(n_nodes, dtype=np.int64)
    for s in range(NBS):
        for c, b in enumerate(slot_blocks[s]):
            if b < 0:
                continue
            lo, hi = b * 128, min((b + 1) * 128, n_nodes)
            nodes = order[lo:hi]
            core_of[nodes] = c
            loc_of[nodes] = s * 128 + np.arange(hi - lo)
    trow = core_of * PR + loc_of  # table row of each node

    # chunk plan (same for all cores): per slot, dc x K chunks, n_idx %128
    MAXI = 3968  # descriptor-ring limit at 64KB DMA scratch
    chunks = []  # (slot, d0, dc, K, ioff)
    ioff = 0
    for s in range(NBS):
        K = slot_K[s]
        dc = 128
        while dc * K > MAXI:
            dc //= 2
        assert dc >= 32
        for d0 in range(0, 128, dc):
            cK = range_K(s, d0, dc)
            while (dc * cK) % 128:
                cK += 1
            chunks.append((s, d0, dc, cK, ioff))
            ioff += dc * cK
    TOT = int(ioff)
    assert TOT % 16 == 0

    # fill slot arrays: edge (src -> dst): slot (k, lane) of dst's chunk
    slots = np.full((n_cores, TOT), ZROW, dtype=np.int64)
    dst_c = core_of[col]
    dst_l = loc_of[col]
    sort_k = np.argsort(dst_c * n_nodes + dst_l, kind="stable")
    sc, sd, ss = dst_c[sort_k], dst_l[sort_k], trow[row[sort_k]]
    key = sc * n_nodes + sd
    first = np.r_[True, key[1:] != key[:-1]]
    run_start = np.maximum.accumulate(np.where(first, np.arange(key.size), 0))
    rank = np.arange(key.size) - run_start  # k within (core, dest)

    # per-dest (slot s, lane within 128): base offset + k-major position
    dest_chunk_base = np.zeros(PT, dtype=np.int64)
    dest_chunk_dc = np.zeros(PT, dtype=np.int64)
    dest_lane = np.zeros(PT, dtype=np.int64)
    for (s, d0, dc, cK, io) in chunks:
        idx = s * 128 + d0 + np.arange(dc)
        dest_chunk_base[idx] = io
        dest_chunk_dc[idx] = dc
        dest_lane[idx] = np.arange(dc)
    flat = dest_chunk_base[sd] + rank * dest_chunk_dc[sd] + dest_lane[sd]
    slots[sc, flat] = ss

    # wrap to [128, TOT//16] int16 (dma_gather index format)
    wrapped = np.empty((n_cores, 128, TOT // 16), dtype=np.int16)
    for c in range(n_cores):
        w16 = slots[c].reshape(TOT // 16, 16).T.astype(np.int16)
        wrapped[c] = np.tile(w16, (8, 1))

    # per-core degree arrays over local dest slots (dummies: deg_full=1)
    degf_loc = np.ones((n_cores, PT), dtype=np.float64)
    has_node = np.zeros((n_cores, PT), dtype=bool)
    degf_loc[core_of, loc_of] = deg_full
    has_node[core_of, loc_of] = True

    return dict(
        PT=PT, PR=PR, NTAB=NTAB, ZROW=ZROW, TOT=TOT, NBS=NBS,
        slot_K=slot_K, chunks=chunks,
        core_of=core_of, loc_of=loc_of, trow=trow,
        dinv=dinv, degf_loc=degf_loc, has_node=has_node, gidx=wrapped,
    )


# -------------------- bass kernel builder --------------------
def _build_bass(h, PT, PR, NTAB, TOT, NBS, chunks, repeat=1, collective=True):
    import concourse.bass as bass
    import concourse.bacc as bacc
    import concourse.mybir as mybir
    import concourse.tile as tile

    dt = mybir.dt
    f32, bf16, i16 = dt.float32, dt.bfloat16, dt.int16
    AF = mybir.ActivationFunctionType
    KC = h // 128  # contraction chunks (2)

    nc = bacc.Bacc(dynamic_dma_scratch_size=65536)
    table0_in = nc.declare_dram_parameter("table0", [NTAB, h], bf16, isOutput=False)
    t0f_in = nc.declare_dram_parameter("t0f", [128, KC, PT], bf16, isOutput=False)
    idx_in = nc.declare_dram_parameter("gidx", [128, TOT // 16], i16, isOutput=False)
    dinv_in = nc.declare_dram_parameter("dinv_nm", [128, NBS], f32, isOutput=False)
    dinv2_in = nc.declare_dram_parameter("dinv2_nm", [128, NBS], f32, isOutput=False)
    sqd_in = nc.declare_dram_parameter("sqd", [1, PT], bf16, isOutput=False)
    ident_in = nc.declare_dram_parameter("ident", [128, 128], bf16, isOutput=False)
    W_in = [nc.declare_dram_parameter(nm, [128, KC, h], bf16, isOutput=False)
            for nm in ("W1", "W1_1", "W2", "W3")]
    b_in = [nc.declare_dram_parameter(nm, [1, h], bf16, isOutput=False)
            for nm in ("b1", "b1_1", "b2", "b3")]
    out2_ext = nc.declare_dram_parameter("out2", [PT, h], bf16, isOutput=True)
    out3_ext = nc.declare_dram_parameter("out3", [PT, h], bf16, isOutput=True)

    # chunks grouped by block slot
    by_slot = [[] for _ in range(NBS)]
    for ch in chunks:
        by_slot[ch[0]].append(ch)
    slot_K = {s: max(cK for (_s, _d0, _dc, cK, _io) in by_slot[s])
              for s in range(NBS)}
    # processing order: big K first (the layer tail, which gates the next
    # AllGather, drains through the cheapest slots)
    slot_order = sorted(range(NBS), key=lambda s: -slot_K[s])

    with tile.TileContext(nc) as tc:
        with (
            tc.tile_pool(name="dram", bufs=1, space="DRAM") as dpool,
            tc.tile_pool(name="const", bufs=1) as cpool,
            tc.tile_pool(name="gather", bufs=4) as gpool,
            tc.tile_pool(name="work", bufs=4) as wpool,
            tc.tile_pool(name="psum", bufs=6, space="PSUM") as ppool,
            tc.tile_pool(name="tpsum", bufs=2, space="PSUM") as tppool,
        ):
            # ---- internal DRAM: AG inputs + tables for layers 1,2 ----
            ag_in_r = [
                [dpool.tile([PR, h], bf16, name=f"agin{L}_{r}") for L in range(2)]
                for r in range(repeat)
            ]
            if collective:
                tables_r = [
                    [dpool.tile([NTAB, h], bf16, addr_space="Shared",
                                name=f"table{L}_{r}") for L in range(2)]
                    for r in range(repeat)
                ]
            else:  # timing-study variant: tables fed as plain inputs, no AG
                tin = [
                    nc.declare_dram_parameter(f"tbl{L}", [NTAB, h], bf16,
                                              isOutput=False)
                    for L in range(2)
                ]
                tables_r = [tin for _ in range(repeat)]

            # ---- constants ----
            w_sb = []
            for i in range(4):
                wt = cpool.tile([128, KC, h], bf16, name=f"w{i}")
                nc.sync.dma_start(wt[:], W_in[i][:])
                w_sb.append(wt)
            b_sb = []
            for i in range(4):
                bt = cpool.tile([1, h], bf16, name=f"bv{i}")
                nc.sync.dma_start(bt[:], b_in[i][:])
                b_sb.append(bt)

            dinv_nm = cpool.tile([128, NBS], f32, name="dinv_nm")
            nc.sync.dma_start(dinv_nm[:], dinv_in[:])
            dinv2_nm = cpool.tile([128, NBS], f32, name="dinv2_nm")
            nc.sync.dma_start(dinv2_nm[:], dinv2_in[:])
            sqd_row = cpool.tile([1, PT], bf16, name="sqd_row")
            nc.sync.dma_start(sqd_row[:], sqd_in[:])
            ident = cpool.tile([128, 128], bf16, name="ident")
            nc.sync.dma_start(ident[:], ident_in[:])

            gidx = cpool.tile([128, TOT // 16], i16, name="gidx_sb")
            nc.sync.dma_start(gidx[:], idx_in[:])

            t0f = cpool.tile([128, KC, PT], bf16, name="t0f")
            nc.sync.dma_start(t0f[:], t0f_in[:])

            rg = [list(range(C))]
            zpad = cpool.tile([PR - PT, h], bf16, name="zpad")
            nc.vector.memset(zpad[:], 0.0)

            def mm_into(ps, lhs_pieces, s, wi, start=True):
                """psum[dest, :] = sum_c lhsT_c @ W + sqd x b, for each
                (d0, dc, lhsT) piece of the 128-dest block."""
                for (d0, dc, lhsT) in lhs_pieces:
                    for c in range(KC):
                        nc.tensor.matmul(
                            ps[d0:d0 + dc, :],
                            lhsT=lhsT[:, c, :],
                            rhs=w_sb[wi][:, c, :],
                            start=(start and c == 0),
                            stop=False,
                        )
                nc.tensor.matmul(
                    ps[:],
                    lhsT=sqd_row[0:1, s * 128:(s + 1) * 128],
                    rhs=b_sb[wi][:],
                    start=False,
                    stop=True,
                )

            def process_layer(rep, L, T_cur, T_next):
                """Gather+reduce all blocks of layer L from its table, matmul,
                and emit either the next layer's AG input + transposed local
                tile (L<2) or the two output heads (L==2)."""
                ag_in = ag_in_r[rep]
                if L > 0 and collective:
                    nc.gpsimd.collective_compute(
                        "AllGather",
                        mybir.AluOpType.bypass,
                        replica_groups=rg,
                        ins=[ag_in[L - 1].opt()],
                        outs=[tables_r[rep][L - 1].opt()],
                    )
                table = table0_in if L == 0 else tables_r[rep][L - 1]
                for s in slot_order:
                    pieces = []
                    for (_s, d0, dc, cK, io) in by_slot[s]:
                        n_idx = dc * cK
                        gt = gpool.tile([128, KC, n_idx], bf16, tag="gt",
                                        name=f"gt{rep}_{L}_{s}_{d0}")
                        nc.gpsimd.dma_gather(
                            gt[:],
                            table[:, :],
                            gidx[:, io // 16:(io + n_idx) // 16],
                            n_idx,
                            n_idx,
                            h,
                            transpose=True,
                            single_packet=(n_idx <= 896),
                        )
                        # in-place pair-add tree over K (k-major: last dim is
                        # the packed dest lane -> DVE 2x mode at every level)
                        g4 = gt.rearrange("p c (k d) -> p c k d", d=dc)
                        K = cK
                        while K > 1:
                            lo = K // 2
                            fold = K - lo  # add slots [fold, K) onto [0, lo)
                            nc.vector.tensor_add(
                                g4[:, :, 0:lo, :],
                                g4[:, :, 0:lo, :],
                                g4[:, :, fold:K, :],
                            )
                            K = fold
                        # fold the self-loop term (local feature-major tile)
                        lane0 = s * 128 + d0
                        nc.vector.tensor_add(
                            g4[:, :, 0, :],
                            g4[:, :, 0, :],
                            T_cur[:, :, lane0:lane0 + dc],
                        )
                        pieces.append((d0, dc, g4[:, :, 0, :]))
                    rows = 128
                    if L < 2:
                        ps = ppool.tile([128, h], f32, tag="ps",
                                        name=f"ps{rep}_{L}_{s}")
                        mm_into(ps, pieces, s, L)
                        # T = dinv*relu(dinv*(RW) + b) = relu(dinv^2*psum)
                        tt = wpool.tile([128, h], bf16, tag="tt",
                                        name=f"tt{rep}_{L}_{s}")
                        nc.scalar.activation(
                            tt[:], ps[:], AF.Relu, scale=dinv2_nm[:, s:s + 1]
                        )
                        nc.sync.dma_start(
                            ag_in[L][s * 128:s * 128 + rows, :], tt[:rows, :]
                        )
                        # transpose tt into the next layer's local tile
                        for c in range(KC):
                            tp = tppool.tile([128, 128], bf16, tag="tp",
                                             name=f"tp{rep}_{L}_{s}_{c}")
                            nc.tensor.transpose(
                                tp[:], tt[:, c * 128:(c + 1) * 128], ident[:]
                            )
                            nc.vector.tensor_copy(
                                T_next[:, c, s * 128:(s + 1) * 128], tp[:]
                            )
                    else:
                        ps2 = ppool.tile([128, h], f32, tag="ps",
                                         name=f"ps2_{rep}_{s}")
                        mm_into(ps2, pieces, s, 2)
                        ps3 = ppool.tile([128, h], f32, tag="ps",
                                         name=f"ps3_{rep}_{s}")
                        mm_into(ps3, pieces, s, 3)
                        o2 = wpool.tile([128, h], bf16, tag="hsb",
                                        name=f"o2_{rep}_{s}")
                        nc.scalar.activation(
                            o2[:], ps2[:], AF.Copy, scale=dinv_nm[:, s:s + 1]
                        )
                        nc.sync.dma_start(
                            out2_ext[s * 128:s * 128 + rows, :], o2[:rows, :]
                        )
                        o3 = wpool.tile([128, h], bf16, tag="hsb",
                                        name=f"o3_{rep}_{s}")
                        nc.scalar.activation(
                            o3[:], ps3[:], AF.Copy, scale=dinv_nm[:, s:s + 1]
                        )
                        nc.sync.dma_start(
                            out3_ext[s * 128:s * 128 + rows, :], o3[:rows, :]
                        )

            for rep in range(repeat):
                ag_in = ag_in_r[rep]
                for L in range(2):
                    nc.sync.dma_start(ag_in[L][PT:PR, :], zpad[:])
                T1 = cpool.tile([128, KC, PT], bf16, name=f"T1_{rep}")
                T2 = cpool.tile([128, KC, PT], bf16, name=f"T2_{rep}")
                process_layer(rep, 0, t0f, T1)
                process_layer(rep, 1, T1, T2)
                process_layer(rep, 2, T2, None)

    nc.compile()
    return nc


# -------------------- public entry --------------------
def kernel(x, edge_index, W1, b1, W1_1, b1_1, W2, b2, W3, b3):
    from concourse.bass_utils import run_bass_kernel_spmd

    x = np.asarray(x, dtype=np.float32)
    edge_index = np.asarray(edge_index, dtype=np.int32)
    n_nodes, h = x.shape
    meta = _prep_graph(edge_index, n_nodes, C)
    PT, PR, NTAB, TOT, NBS = (meta["PT"], meta["PR"], meta["NTAB"],
                              meta["TOT"], meta["NBS"])

    key = (n_nodes, h, TOT, tuple(meta["slot_K"]))
    if key not in _KERNEL_CACHE:
        _KERNEL_CACHE[key] = _build_bass(
            h, PT, PR, NTAB, TOT, NBS, meta["chunks"],
        )
    nc = _KERNEL_CACHE[key]

    core_of, loc_of, dinv = meta["core_of"], meta["loc_of"], meta["dinv"]
    degf_loc = meta["degf_loc"]

    # layer-0 table: dinv-scaled x in table-row order, bf16, zero padded
    table0 = np.zeros((NTAB, h), dtype=BF16)
    trow = meta["trow"]
    table0[trow] = (dinv[:, None] * x).astype(BF16)

    # per-core feature-major local tile: t0f[f, c, loc] = table0[row(loc), :]
    KC = h // 128
    t0f_all = np.zeros((C, 128, KC, PT), dtype=BF16)
    for c in range(C):
        rows = table0[c * PR:c * PR + PT]  # [PT, h]
        t0f_all[c] = rows.reshape(PT, KC, 128).transpose(2, 1, 0)

    dinv_loc = 1.0 / np.sqrt(degf_loc)  # [C, PT]
    dinv_nm = dinv_loc.reshape(C, NBS, 128).transpose(0, 2, 1).astype(np.float32)
    dinv2_nm = (dinv_loc ** 2).reshape(C, NBS, 128).transpose(0, 2, 1).astype(np.float32)
    sqd = np.sqrt(degf_loc).astype(BF16)[:, None, :]  # [C, 1, PT]

    Wsb = {}
    for nm, W in (("W1", W1), ("W1_1", W1_1), ("W2", W2), ("W3", W3)):
        Wf = np.ascontiguousarray(W, dtype=np.float32)  # [h, h]
        Wsb[nm] = Wf.reshape(KC, 128, h).transpose(1, 0, 2).astype(BF16)
    bsb = {nm: np.asarray(b, dtype=BF16).reshape(1, h)
           for nm, b in (("b1", b1), ("b1_1", b1_1), ("b2", b2), ("b3", b3))}
    ident = np.eye(128, dtype=BF16)

    in_maps = []
    for c in range(C):
        m = {
            "table0": table0,
            "t0f": t0f_all[c],
            "gidx": np.ascontiguousarray(meta["gidx"][c]),
            "dinv_nm": dinv_nm[c],
            "dinv2_nm": dinv2_nm[c],
            "sqd": sqd[c],
            "ident": ident,
        }
        m.update(Wsb)
        m.update(bsb)
        in_maps.append(m)

    global LAST_RESULTS
    LAST_RESULTS = run_bass_kernel_spmd(nc, in_maps, core_ids=list(range(C)))
    res = LAST_RESULTS.results

    out2 = np.empty((n_nodes, h), dtype=np.float32)
    out3 = np.empty((n_nodes, h), dtype=np.float32)
    for c in range(C):
        sel = core_of == c
        out2[sel] = res[c]["out2"][loc_of[sel]].astype(np.float32)
        out3[sel] = res[c]["out3"][loc_of[sel]].astype(np.float32)
    return out2, out3
